# revision 38
# baseline (speedup 1.0000x reference)
"""Trainium2 Bass kernel for MultiHeadAttention with relative position bias.

Problem: B=512, L=32, E=2048, H=32, D=64 (nn_MultiHeadAttention_69380901699750)

  q = x@wq.T+bq ; k = x@wk.T+bk ; v = x@wv.T+bv        (per-head [L,D])
  S[b,h] = scale * q_bh @ k_bh.T + q_bh @ rel[h].T     (rel[h][j,:] = rpe[h-j+31,:])
  out = softmax(S) @ v_bh  ->  reshape -> @ wo.T + bo

Data-parallel over batch across 8 cores (64 batches = 2048 tokens per core).

Per-core design (v2 — fp8 DoubleRow GEMMs):
  All four E x E GEMMs run as 3-term error-compensated fp8e4m3 DoubleRow
  matmuls (x_hi@W_hi + x_hi@W_lo + x_lo@W_hi, weights pre-scaled by 64 on
  the host, 1/64 folded into the psum eviction).  DoubleRow packs K=256
  per instruction at 0.5 cycles/output-row, so each GEMM costs 75% of its
  bf16 cycle count.  Measured rel RMS error of a lone compensated GEMM is
  ~1.3e-3, slightly better than bf16.

  A. x arrives as a packed uint16 tensor (byte0 = e4m3(x), byte1 =
     e4m3(x - hi)); DMA-XBAR-transposed straight into SBUF (no PE
     transposes, no DVE casts).  fp8 operand views are strided bitcasts.
     V = x@wv'^T/64 + bv evicted bf16 and scattered to DRAM in a
     block-diagonal-friendly layout.
  B. Q^T and K'^T (K' = scale*k + rel[h]) per 128-row tile m: lhsT = host
     DR-packed W chunks, rhs = fp8 xT views.  Q evicts to stq[m] bf16.
     K' evicts (ACT, scale+bias) scatter directly into kbd[m], a
     [128, 64*64] block-diagonal lhsT image (2 heads per tile), rel added
     in place by DVE.  Attention rounds (hq, b0) interleave into the proj
     loop: MM1 is 2 block-diag matmuls per b16 (4 (b,h) pairs per 64
     cycles); exp on ACT; group-sums via ONE block-ones matmul that
     broadcasts sums to all 32 partitions of each group; DVE reciprocal +
     multiply; MM2 is 1 block-diag matmul per (b16, head-pair) reading
     ptn at partition bases {0,64} and a zero-padded vbdr tile loaded
     from DRAM.  O evicts as (hi, lo) fp8 pair packed uint16 (ACT hi,
     DVE lo) forming the DR-packed lhsT for phase C.
  C. out = O @ wo'^T/64 + bo, n-major with wo chunks streamed (no
     resident wo), 3-term fp8 DR like the other GEMMs.

PE cycle count ~1.67M (vs 2.29M baseline bf16); target ~700 us.
"""

import os
import sys

for _p in ("/opt/trn_rl_repo", "/root/.axon_site/_ro/trn_rl_repo"):
    if os.path.isdir(_p) and _p not in sys.path:
        sys.path.append(_p)

import numpy as np
import ml_dtypes

import concourse.bass as bass
import concourse.mybir as mybir
import concourse.tile as tile
from concourse import bacc
from concourse import bass_utils

F32 = mybir.dt.float32
BF16 = mybir.dt.bfloat16
FP8 = mybir.dt.float8e4
U16 = mybir.dt.uint16
BF = ml_dtypes.bfloat16
E4M3 = ml_dtypes.float8_e4m3

N_CORES = 8
B, L, E, H, D = 512, 32, 2048, 32, 64
BS = B // N_CORES          # 64 batches per core
T = BS * L                 # 2048 tokens per core
P = 128
KT = E // P                # 16 contraction tiles (128)
CT = E // 256              # 8 contraction tiles (256, DoubleRow)
MT = T // P                # 16 row tiles
NT = 4                     # 512-wide output column tiles
NW = 512
SCALE = D ** -0.5
WS = 64.0                  # host weight prescale (folded out at eviction)
IWS = 1.0 / WS

HQ = H // 4                # 8 head-quad groups; 4 rounds (b0) each

Ident = mybir.ActivationFunctionType.Identity
Exp = mybir.ActivationFunctionType.Exp
DR = mybir.MatmulPerfMode.DoubleRow
MUL = mybir.AluOpType.mult
ADD = mybir.AluOpType.add

# (x-view, w-view) per compensation term: 0 = hi, 1 = lo
TERMS = ((0, 0), (0, 1), (1, 0))


def build_kernel(nc: bass.Bass, ph: int = 5):
    """ph: 1=A only, 2=+projections, 3=+MM1/exp, 4=+tails, 5=+C (full)."""
    f = nc.dram_tensor
    xpk_d = f("xpk", (T, E), U16, kind="ExternalInput").ap()
    wqp_d = f("wqp", (E, 2 * E), FP8, kind="ExternalInput").ap()
    wkp_d = f("wkp", (E, 2 * E), FP8, kind="ExternalInput").ap()
    wvp_d = f("wvp", (P, NT * 16384), FP8, kind="ExternalInput").ap()
    wop_d = f("wop", (P, NT * 16384), FP8, kind="ExternalInput").ap()
    bqk_d = f("bqk", (P, 2 * KT), F32, kind="ExternalInput").ap()
    bvf_d = f("bvf", (P, E), BF16, kind="ExternalInput").ap()
    bof_d = f("bof", (P, E), BF16, kind="ExternalInput").ap()
    relt_d = f("relt", (P, NW), BF16, kind="ExternalInput").ap()
    bonesb_d = f("bonesb", (P, P), BF16, kind="ExternalInput").ap()
    out_d = f("out", (T, E), F32, kind="ExternalOutput").ap()

    with tile.TileContext(nc) as tc:
        with (
            tc.tile_pool(name="dram", bufs=1, space="DRAM") as dram,
            tc.tile_pool(name="const", bufs=1) as const,
            tc.tile_pool(name="otp", bufs=1) as otp,
        ):
            relt = const.tile([P, NW], BF16)
            nc.gpsimd.dma_start(relt[:], relt_d[:])
            bqk = const.tile([P, 2 * KT], F32)
            nc.gpsimd.dma_start(bqk[:], bqk_d[:])
            bonesb = const.tile([P, P], BF16)
            nc.gpsimd.dma_start(bonesb[:], bonesb_d[:])

            # V in DRAM, laid out for block-diagonal round loads:
            # vd4[(hq//2)*128 + b0*32 + j, (hq%2)*4096 + hl*1024 + b16*64 + d]
            #   = V[b0*16+b16, j, 256*hq + 64*hl + d]
            vd4 = dram.tile([512, 8192], BF16)
            vd6 = vd4[:].rearrange(
                "(nn b0 j) (hq2 hl b4 pb d) -> nn b0 b4 pb j hq2 hl d",
                nn=NT, b0=4, j=32, hq2=2, hl=4, b4=4, pb=4, d=64)

            # O^T packed fp8 pair (byte0=hi, byte1=lo), DR-layout lhsT for C
            otu = otp.tile([P, KT * T], U16)
            otf = otu[:].bitcast(FP8).rearrange(
                "p (r t w) -> p r w t", r=KT, w=2)

            # Pools that live from phase A through phase C (attention tail
            # interleaves with early C blocks).  One shared PSUM pool (gps)
            # serves A's psv, B's pps and C's fps.
            with (
                tc.tile_pool(name="attp", bufs=1) as att_pool,
                tc.tile_pool(name="wqk", bufs=1) as wqk_pool,
                tc.tile_pool(name="gps", bufs=3, space="PSUM") as gps_p,
                tc.tile_pool(name="psb", bufs=1, space="PSUM") as psb_p,
                tc.tile_pool(name="pso", bufs=2, space="PSUM") as pso_p,
            ):
                stq = {}
                kbd = {}
                w_pre = {}
                init_cnt = {"kbd": 0, "vbd": 0}
                hold = {}

                def wload(which, m):
                    if (which, m) in w_pre:
                        return w_pre.pop((which, m))
                    wt = wqk_pool.tile([P, 2 * E], FP8, tag="w", bufs=3,
                                       name="wt")
                    nc.sync.dma_start(
                        wt[:],
                        (wqp_d if which == 0 else wkp_d)
                        [m * P:(m + 1) * P, :])
                    return wt

                def wprefetch(which, m):
                    w_pre[(which, m)] = wload(which, m)

                def mm1_round(hq, b0):
                    # vbdr[64*c2+32*u+j, b16*128 + u*64 + d] = V[b,j,...]
                    vbdr = att_pool.tile([P, 2048], BF16, tag="vbd",
                                         bufs=4, name="vbdr")
                    vb3 = vbdr[:].rearrange(
                        "p (b16 u d) -> p b16 u d", u=2, d=64)
                    if init_cnt["vbd"] < 4:
                        init_cnt["vbd"] += 1
                        nc.gpsimd.memset(vbdr[:], 0)
                    rbase = (hq // 2) * 128 + b0 * 32
                    for u in range(2):
                        for c2 in range(2):
                            hl = 2 * c2 + u
                            co = (hq % 2) * 4096 + hl * 1024
                            nc.scalar.dma_start(
                                vb3[64 * c2 + 32 * u:
                                    64 * c2 + 32 * u + 32, :, u, :],
                                vd4[rbase:rbase + 32, co:co + 1024]
                                .rearrange("j (b16 d) -> j b16 d", d=64),
                            )
                    pss = hold["pss_p"].tile([P, NW], F32, tag="ss",
                                             name="pss")
                    for b16 in range(16):
                        b = b0 * 16 + b16
                        for pi in range(2):
                            m = 2 * hq + pi
                            nc.tensor.matmul(
                                pss[64 * pi:64 * pi + 64,
                                    32 * b16:32 * b16 + 32],
                                kbd[m][:, b * 64:b * 64 + 64],
                                stq[m][:, b * L:(b + 1) * L],
                                start=True,
                                stop=True,
                                tile_position=(0, 64 * pi),
                            )
                    ptt = att_pool.tile([P, NW], BF16, tag="ptt",
                                        bufs=4, name="ptt")
                    nc.scalar.activation(ptt[:], pss[:], Exp, bias=0.0)
                    return ptt, vbdr

                def tails_head(hq, b0, rounds):
                    ptt, vbdr = rounds
                    psb = psb_p.tile([P, NW], F32, tag="bc", name="psb")
                    nc.tensor.matmul(psb[:], bonesb[:], ptt[:],
                                     start=True, stop=True)
                    rbc = att_pool.tile([P, NW], F32, tag="rbc",
                                        bufs=2, name="rbc")
                    nc.vector.reciprocal(rbc[:], psb[:])
                    ptn = att_pool.tile([P, NW], BF16, tag="ptn",
                                        bufs=2, name="ptn")
                    nc.vector.tensor_mul(ptn[:], ptt[:], rbc[:])
                    return ptn

                def tails_mm2(hq, b0, rounds, ptn):
                    ptt, vbdr = rounds
                    for c2 in range(2):
                        pso = pso_p.tile([P, NW], F32, tag="so",
                                         name="pso")
                        for b16 in range(16):
                            nc.tensor.matmul(
                                pso[:, 32 * b16:32 * b16 + 32],
                                vbdr[64 * c2:64 * c2 + 64,
                                     b16 * P:(b16 + 1) * P],
                                ptn[64 * c2:64 * c2 + 64,
                                    32 * b16:32 * b16 + 32],
                                start=True,
                                stop=True,
                                tile_position=(64 * c2, 0),
                            )
                        rt = 2 * hq + c2
                        hi = otf[:, rt, 0, b0 * NW:(b0 + 1) * NW]
                        lo = otf[:, rt, 1, b0 * NW:(b0 + 1) * NW]
                        with nc.allow_low_precision(
                            reason="fp8 hi/lo split keeps 2^-12 residual"
                        ):
                            nc.scalar.activation(hi, pso[:], Ident,
                                                 bias=0.0)
                            nc.vector.scalar_tensor_tensor(
                                lo, hi, -1.0, pso[:], op0=MUL, op1=ADD)

                def tails(hq, b0, rounds):
                    ptn = tails_head(hq, b0, rounds)
                    tails_mm2(hq, b0, rounds, ptn)

                tail_r = {}
                with tc.tile_pool(name="bigp", bufs=1) as bigp:
                    bigu = bigp.tile([P, KT * T], U16)
                    bigu3 = bigu[:].rearrange("p (k t) -> p k t", k=KT)
                    # fp8 views: [p][k 16][w 2][t 2048]
                    bigf = bigu[:].bitcast(FP8).rearrange(
                        "p (k t w) -> p k w t", k=KT, w=2)

                    # ---------------- Phase A ----------------
                    with tc.tile_pool(name="apool", bufs=1) as apool:
                        bvf = apool.tile([P, E], BF16)
                        nc.gpsimd.dma_start(bvf[:], bvf_d[:])

                        def wvload(n, nsplit=1):
                            wvn = apool.tile([P, 16384], FP8, tag="wv",
                                             bufs=2, name="wvn")
                            w = 16384 // nsplit
                            for h in range(nsplit):
                                nc.scalar.dma_start(
                                    wvn[:, h * w:(h + 1) * w],
                                    wvp_d[:, n * 16384 + h * w:
                                          n * 16384 + (h + 1) * w])
                            return wvn

                        def vproj(n, m, wvn):
                            wvn3 = wvn[:].rearrange(
                                "p (tw c i ncl) -> p tw c i ncl",
                                tw=2, c=CT, i=2)
                            psv = gps_p.tile([P, NW], F32, tag="g",
                                             name="psv")
                            idx = 0
                            for (xw, tw) in TERMS:
                                for c in range(CT):
                                    nc.tensor.matmul(
                                        psv[:],
                                        bigf[:, 2 * c:2 * c + 2, xw,
                                             m * P:(m + 1) * P],
                                        wvn3[:, tw, c],
                                        start=(idx == 0),
                                        stop=(idx == 3 * CT - 1),
                                        perf_mode=DR,
                                    )
                                    idx += 1
                            vev = apool.tile([P, NW], BF16, tag="vev",
                                             bufs=4, name="vev")
                            nc.vector.scalar_tensor_tensor(
                                vev[:], psv[:], IWS,
                                bvf[:, n * NW:(n + 1) * NW],
                                op0=MUL, op1=ADD,
                            )
                            # scatter (b0 = m//4, b16 = 4*(m%4) + p//32)
                            nc.gpsimd.dma_start(vd6[n, m // 4, m % 4],
                                                vev[:])

                        wvn0 = apool.tile([P, 16384], FP8, tag="wv",
                                          bufs=2, name="wvn0")
                        # tile 0 in e-halves: first vproj matmuls can
                        # start after the first half + first wv chunk
                        nc.sync.dma_start_transpose(
                            bigu3[:, 0:8, 0:P], xpk_d[0:P, 0:E // 2])
                        nc.scalar.dma_start(wvn0[:, 0:4096],
                                            wvp_d[:, 0:4096])
                        nc.sync.dma_start_transpose(
                            bigu3[:, 8:KT, 0:P], xpk_d[0:P, E // 2:E])
                        nc.scalar.dma_start(wvn0[:, 4096:8192],
                                            wvp_d[:, 4096:8192])
                        for tt in range(1, 3):
                            nc.sync.dma_start_transpose(
                                bigu3[:, :, tt * P:(tt + 1) * P],
                                xpk_d[tt * P:(tt + 1) * P, :])
                            h2 = slice((tt + 1) * 4096, (tt + 2) * 4096)
                            nc.scalar.dma_start(wvn0[:, h2], wvp_d[:, h2])
                        wvn1 = apool.tile([P, 16384], FP8, tag="wv", bufs=2,
                                          name="wvn1")
                        # interleave n=0 and n=1 passes: the transpose DMA
                        # deficit amortizes over twice the PE work
                        OFF = 6
                        for m in range(MT):
                            if m < 4:
                                nc.scalar.dma_start(
                                    wvn1[:, m * 4096:(m + 1) * 4096],
                                    wvp_d[:, 16384 + m * 4096:
                                          16384 + (m + 1) * 4096])
                            if m + 3 < MT:
                                nc.sync.dma_start_transpose(
                                    bigu3[:, :, (m + 3) * P:(m + 4) * P],
                                    xpk_d[(m + 3) * P:(m + 4) * P, :])
                            vproj(0, m, wvn0)
                            if m >= OFF:
                                vproj(1, m - OFF, wvn1)
                        wvn2 = apool.tile([P, 16384], FP8, tag="wv",
                                          bufs=2, name="wvn2")
                        for m in range(MT - OFF, MT):
                            if m - (MT - OFF) < 4:
                                h = m - (MT - OFF)
                                nc.scalar.dma_start(
                                    wvn2[:, h * 4096:(h + 1) * 4096],
                                    wvp_d[:, 32768 + h * 4096:
                                          32768 + (h + 1) * 4096])
                            vproj(1, m, wvn1)
                        wprefetch(0, 0)
                        wvn3 = apool.tile([P, 16384], FP8, tag="wv",
                                          bufs=2, name="wvn3")
                        for m in range(MT):
                            if m < 4:
                                nc.scalar.dma_start(
                                    wvn3[:, m * 4096:(m + 1) * 4096],
                                    wvp_d[:, 49152 + m * 4096:
                                          49152 + (m + 1) * 4096])
                            vproj(2, m, wvn2)
                        for m in range(MT):
                            vproj(3, m, wvn3)
                        wprefetch(1, 0)

                    # ---------------- Phase B ----------------
                    with (
                        tc.tile_pool(name="stq", bufs=1) as stq_pool,
                        tc.tile_pool(name="kbd", bufs=1) as kbd_pool,
                        tc.tile_pool(name="pss", bufs=2,
                                     space="PSUM") as pss_p,
                    ):
                        hold["pss_p"] = pss_p

                        def proj(which, m):
                            wt = wload(which, m)
                            wt3 = wt[:].rearrange(
                                "p (tw c i mc) -> p tw c i mc",
                                tw=2, c=CT, i=2)
                            if which == 0:
                                st = stq_pool.tile([P, T], BF16, tag="stq",
                                                   bufs=3, name="st")
                                stq[m] = st
                            else:
                                kb = kbd_pool.tile([P, 64 * BS], BF16,
                                                   tag="kbd", bufs=3,
                                                   name="kb")
                                kbd[m] = kb
                                kb3 = kb[:].rearrange("p (b z) -> p b z",
                                                      b=BS)
                                if init_cnt["kbd"] < 3:
                                    init_cnt["kbd"] += 1
                                    nc.gpsimd.memset(kb[:], 0)
                            for n in range(NT):
                                pps = gps_p.tile([P, NW], F32, tag="g",
                                                 name="pps")
                                idx = 0
                                for (xw, tw) in TERMS:
                                    for c in range(CT):
                                        nc.tensor.matmul(
                                            pps[:],
                                            wt3[:, tw, c],
                                            bigf[:, 2 * c:2 * c + 2, xw,
                                                 n * NW:(n + 1) * NW],
                                            start=(idx == 0),
                                            stop=(idx == 3 * CT - 1),
                                            perf_mode=DR,
                                        )
                                        idx += 1
                                bs = slice(16 * n, 16 * (n + 1))
                                if which == 0:
                                    nc.scalar.activation(
                                        st[:, n * NW:(n + 1) * NW], pps[:],
                                        Ident, bias=bqk[:, m:m + 1],
                                        scale=IWS)
                                else:
                                    # scatter-evict into block-diag kbd
                                    for hh in range(2):
                                        pr = slice(64 * hh, 64 * hh + 64)
                                        dst = kb3[pr, bs,
                                                  32 * hh:32 * hh + 32]
                                        nc.scalar.activation(
                                            dst,
                                            pps[pr, :].rearrange(
                                                "p (b j) -> p b j", j=L),
                                            Ident,
                                            bias=bqk[pr,
                                                     KT + m:KT + m + 1],
                                            scale=IWS)
                                        rel3 = (relt[pr,
                                                     m * L:(m + 1) * L]
                                                .unsqueeze(1)
                                                .broadcast_to([64, 16, L]))
                                        nc.vector.tensor_add(dst, dst,
                                                             rel3)

                        def proj_pair(g):
                            prev = []
                            if g > 0:
                                prev = [(g - 1, b0) for b0 in range(4)]
                            rnd = [None] * 4
                            wprefetch(0, 2 * g)
                            wprefetch(1, 2 * g)
                            if ph >= 2:
                                proj(0, 2 * g)
                            wprefetch(0, 2 * g + 1)
                            if prev and ph >= 3:
                                rnd[0] = mm1_round(*prev[0])
                            if ph >= 2:
                                proj(1, 2 * g)
                            wprefetch(1, 2 * g + 1)
                            if prev:
                                if ph >= 4:
                                    tails(*prev[0], rnd[0])
                                if ph >= 3:
                                    rnd[1] = mm1_round(*prev[1])
                            if ph >= 2:
                                proj(0, 2 * g + 1)
                            if prev:
                                if ph >= 4:
                                    tails(*prev[1], rnd[1])
                                if ph >= 3:
                                    rnd[2] = mm1_round(*prev[2])
                            if ph >= 2:
                                proj(1, 2 * g + 1)
                            if prev:
                                if ph >= 4:
                                    tails(*prev[2], rnd[2])
                                if ph >= 3:
                                    rnd[3] = mm1_round(*prev[3])
                                    if ph >= 4:
                                        tails(*prev[3], rnd[3])

                        def wp0(which, m):
                            # first-pair prefetches were issued in phase A
                            pass

                        for g in range(HQ):
                            if g == 0:
                                # (0,0) and (1,0) already prefetched in A
                                saved = w_pre.copy()
                            proj_pair(g)

                        # att group 7: MM1 rounds here (pss still open);
                        # tails interleave with early phase-C blocks
                        if ph >= 3:
                            for b0 in range(4):
                                tail_r[b0] = mm1_round(7, b0)

                # ---------------- Phase C ----------------
                with tc.tile_pool(name="cpool", bufs=1) as cpool:
                    bof = cpool.tile([P, E], BF16)
                    nc.gpsimd.dma_start(bof[:], bof_d[:])

                    def woload(n, nsplit=1):
                        won = cpool.tile([P, 16384], FP8, tag="wo", bufs=2,
                                         name="won")
                        w = 16384 // nsplit
                        for h in range(nsplit):
                            nc.scalar.dma_start(
                                won[:, h * w:(h + 1) * w],
                                wop_d[:, n * 16384 + h * w:
                                      n * 16384 + (h + 1) * w])
                        return won

                    def cblock(n, m, won3, mid=None):
                        fps = gps_p.tile([P, NW], F32, tag="g", name="fps")
                        idx = 0
                        for (xw, tw) in TERMS:
                            for c in range(CT):
                                nc.tensor.matmul(
                                    fps[:],
                                    otf[:, 2 * c:2 * c + 2, xw,
                                        m * P:(m + 1) * P],
                                    won3[:, tw, c],
                                    start=(idx == 0),
                                    stop=(idx == 3 * CT - 1),
                                    perf_mode=DR,
                                )
                                idx += 1
                        fout = cpool.tile([P, NW], F32, tag="fo", bufs=3,
                                          name="fout")
                        pieces = 2 if (n == NT - 1 and m == MT - 1) else 1
                        w = NW // pieces
                        for h in range(pieces):
                            cs = slice(h * w, (h + 1) * w)
                            nc.vector.scalar_tensor_tensor(
                                fout[:, cs], fps[:, cs], IWS,
                                bof[:, n * NW + h * w:n * NW + (h + 1) * w],
                                op0=MUL, op1=ADD)
                            nc.sync.dma_start(
                                out_d[m * P:(m + 1) * P,
                                      n * NW + h * w:n * NW + (h + 1) * w],
                                fout[:, cs])

                    tail_h = {}
                    if ph >= 5:
                        won = woload(0, nsplit=4)
                        wonn = woload(1)
                        for n in range(NT):
                            won3 = won[:].rearrange(
                                "p (tw c i ncl) -> p tw c i ncl",
                                tw=2, c=CT, i=2)
                            for m in range(MT):
                                # att7 tails interleave with early C:
                                # heads run one round ahead of their MM2;
                                # mm2(m) must precede cblock(0, m) (its
                                # c'=7 chunk reads rt 14/15 at b0=m).
                                if n == 0 and ph >= 4:
                                    if m == 0:
                                        tail_h[0] = tails_head(
                                            7, 0, tail_r[0])
                                        tail_h[1] = tails_head(
                                            7, 1, tail_r[1])
                                    elif m in (2, 3):
                                        tail_h[m] = tails_head(
                                            7, m, tail_r[m])
                                    if m < 4:
                                        tails_mm2(7, m, tail_r[m],
                                                  tail_h[m])
                                cblock(n, m, won3)
                            won = wonn
                            if n + 2 < NT:
                                wonn = woload(n + 2)
                    elif ph >= 4:
                        for b0 in range(4):
                            tails(7, b0, tail_r[b0])
    return nc


def host_prep(wq, bq, wk, bk, wv, bv, wo, bo, rel_pos_enc):
    """Shared (core-replicated) input tensors, laid out for the kernel."""
    def split8(w):
        hi = w.astype(E4M3)
        lo = (w - hi.astype(np.float32)).astype(E4M3)
        return hi, lo

    def pack_lhst(w):
        # w: [e_out, e_in]; returns (E, 2E) fp8:
        # wp[m*128+p, tw*2048 + c*256 + i*128 + mc] = Wtw^T[256c+128i+p, 128m+mc]
        hi, lo = split8(WS * w)
        wt = np.stack([hi.T, lo.T])           # [tw, e_in, e_out]
        tmp = np.ascontiguousarray(
            wt.reshape(2, CT, 2, P, KT, P).transpose(4, 3, 0, 1, 2, 5))
        return tmp.reshape(E, 2 * E)

    def pack_rhs(w):
        # w: [e_out, e_in]; returns (128, NT*16384) fp8:
        # wp[p, n*16384 + tw*8192 + c*1024 + i*512 + ncl]
        #   = Wtw^T[256c+128i+p, 512n+ncl]
        hi, lo = split8(WS * w)
        wt = np.stack([hi.T, lo.T])           # [tw, e_in, e_out]
        tmp = np.ascontiguousarray(
            wt.reshape(2, CT, 2, P, NT, NW).transpose(3, 4, 0, 1, 2, 5))
        return tmp.reshape(P, NT * 16384)

    wqp = pack_lhst(wq)
    wkp = pack_lhst(SCALE * wk)
    wvp = pack_rhs(wv)
    wop = pack_rhs(wo)

    bqk = np.zeros((P, 2 * KT), np.float32)
    bqk[:, :KT] = bq.reshape(KT, P).T
    bqk[:, KT:] = (SCALE * bk).reshape(KT, P).T
    bvf = np.ascontiguousarray(np.broadcast_to(bv, (P, E))).astype(BF)
    bof = np.ascontiguousarray(np.broadcast_to(bo, (P, E))).astype(BF)

    # relT[64c+d, 32m+j] = rel_pos_enc[(2m+c) - j + 31, d]
    relt = np.zeros((P, NW), np.float32)
    j = np.arange(L)
    for m in range(KT):
        for c in range(2):
            h = 2 * m + c
            blk = rel_pos_enc[h - j + (L - 1), :]        # [j, d]
            relt[64 * c:64 * c + 64, 32 * m:32 * m + 32] = blk.T
    relt = relt.astype(BF)

    bonesb = np.zeros((P, P), np.float32)
    for g in range(4):
        bonesb[32 * g:32 * g + 32, 32 * g:32 * g + 32] = 1
    bonesb = bonesb.astype(BF)

    return dict(
        wqp=wqp, wkp=wkp, wvp=wvp, wop=wop, bqk=bqk, bvf=bvf, bof=bof,
        relt=relt, bonesb=bonesb,
    )


def pack_x(x_core):
    # x_core: [T, E] f32 -> packed uint16 (byte0 = hi, byte1 = lo)
    hi = x_core.astype(E4M3)
    lo = (x_core - hi.astype(np.float32)).astype(E4M3)
    return (hi.view(np.uint8).astype(np.uint16)
            | (lo.view(np.uint8).astype(np.uint16) << 8))


_CACHE = {}


def _get_nc():
    if "nc" not in _CACHE:
        nc = bacc.Bacc(
            "TRN2",
            target_bir_lowering=False,
            debug=False,
            enable_asserts=False,
            num_devices=N_CORES,
        )
        build_kernel(nc, ph=int(os.environ.get("KPH", "5")))
        nc.compile()
        _CACHE["nc"] = nc
    return _CACHE["nc"]


def kernel(x, wq, bq, wk, bk, wv, bv, wo, bo, rel_pos_enc, _return_maps=False):
    x = np.asarray(x, dtype=np.float32)
    shared = host_prep(
        np.asarray(wq, np.float32), np.asarray(bq, np.float32),
        np.asarray(wk, np.float32), np.asarray(bk, np.float32),
        np.asarray(wv, np.float32), np.asarray(bv, np.float32),
        np.asarray(wo, np.float32), np.asarray(bo, np.float32),
        np.asarray(rel_pos_enc, np.float32),
    )
    in_maps = []
    for c in range(N_CORES):
        m = dict(shared)
        m["xpk"] = pack_x(
            np.ascontiguousarray(x[c * BS:(c + 1) * BS].reshape(T, E)))
        in_maps.append(m)
    if _return_maps:
        return in_maps

    nc = _get_nc()
    res = bass_utils.run_bass_kernel_spmd(
        nc, in_maps, core_ids=list(range(N_CORES)), trace=False
    )
    out = np.concatenate(
        [res.results[c]["out"].reshape(BS, L, E) for c in range(N_CORES)],
        axis=0,
    )
    return out.astype(np.float32)


if __name__ == "__main__":
    rng = np.random.default_rng(0)
    ins = {
        "x": rng.standard_normal((B, L, E), dtype=np.float32),
        "wq": rng.standard_normal((E, E), dtype=np.float32) * 0.02,
        "bq": np.zeros(E, np.float32),
        "wk": rng.standard_normal((E, E), dtype=np.float32) * 0.02,
        "bk": np.zeros(E, np.float32),
        "wv": rng.standard_normal((E, E), dtype=np.float32) * 0.02,
        "bv": np.zeros(E, np.float32),
        "wo": rng.standard_normal((E, E), dtype=np.float32) * 0.02,
        "bo": np.zeros(E, np.float32),
        "rel_pos_enc": rng.standard_normal((2 * L - 1, D),
                                           dtype=np.float32),
    }

    def np_ref(x, wq, bq, wk, bk, wv, bv, wo, bo, rel_pos_enc):
        b, l, e = x.shape
        h, d = H, D
        q = (x @ wq.T + bq).reshape(b, l, h, d).transpose(0, 2, 1, 3)
        k = (x @ wk.T + bk).reshape(b, l, h, d).transpose(0, 2, 1, 3)
        v = (x @ wv.T + bv).reshape(b, l, h, d).transpose(0, 2, 1, 3)
        scores = np.einsum("bhid,bhjd->bhij", q, k) * (d ** -0.5)
        pos = np.arange(l)
        rel_idx = pos[:, None] - pos[None, :] + (l - 1)
        rel_pos = rel_pos_enc[rel_idx]
        scores = scores + np.einsum("bhid,hjd->bhij", q, rel_pos)
        scores -= scores.max(axis=-1, keepdims=True)
        attn = np.exp(scores)
        attn /= attn.sum(axis=-1, keepdims=True)
        out = np.einsum("bhij,bhjd->bhid", attn, v)
        out = out.transpose(0, 2, 1, 3).reshape(b, l, e)
        return out @ wo.T + bo

    nb = int(os.environ.get("KNB", "64"))  # batches to check
    out = kernel(**ins)
    exp = np_ref(**{k: (v[:nb] if k == "x" else v) for k, v in ins.items()})
    err = out[:nb] - exp
    rel = np.linalg.norm(err) / np.linalg.norm(exp)
    print(f"kernel out: {out.shape} rel err (first {nb} batches): {rel:.6e}")


# revision 40
# speedup vs baseline: 1.0038x; 1.0038x over previous
"""Trainium2 Bass kernel for MultiHeadAttention with relative position bias.

Problem: B=512, L=32, E=2048, H=32, D=64 (nn_MultiHeadAttention_69380901699750)

  q = x@wq.T+bq ; k = x@wk.T+bk ; v = x@wv.T+bv        (per-head [L,D])
  S[b,h] = scale * q_bh @ k_bh.T + q_bh @ rel[h].T     (rel[h][j,:] = rpe[h-j+31,:])
  out = softmax(S) @ v_bh  ->  reshape -> @ wo.T + bo

Data-parallel over batch across 8 cores (64 batches = 2048 tokens per core).

Per-core design (v2 — fp8 DoubleRow GEMMs):
  All four E x E GEMMs run as 3-term error-compensated fp8e4m3 DoubleRow
  matmuls (x_hi@W_hi + x_hi@W_lo + x_lo@W_hi, weights pre-scaled by 64 on
  the host, 1/64 folded into the psum eviction).  DoubleRow packs K=256
  per instruction at 0.5 cycles/output-row, so each GEMM costs 75% of its
  bf16 cycle count.  Measured rel RMS error of a lone compensated GEMM is
  ~1.3e-3, slightly better than bf16.

  A. x arrives as a packed uint16 tensor (byte0 = e4m3(x), byte1 =
     e4m3(x - hi)); DMA-XBAR-transposed straight into SBUF (no PE
     transposes, no DVE casts).  fp8 operand views are strided bitcasts.
     V = x@wv'^T/64 + bv evicted bf16 and scattered to DRAM in a
     block-diagonal-friendly layout.
  B. Q^T and K'^T (K' = scale*k + rel[h]) per 128-row tile m: lhsT = host
     DR-packed W chunks, rhs = fp8 xT views.  Q evicts to stq[m] bf16.
     K' evicts (ACT, scale+bias) scatter directly into kbd[m], a
     [128, 64*64] block-diagonal lhsT image (2 heads per tile), rel added
     in place by DVE.  Attention rounds (hq, b0) interleave into the proj
     loop: MM1 is 2 block-diag matmuls per b16 (4 (b,h) pairs per 64
     cycles); exp on ACT; group-sums via ONE block-ones matmul that
     broadcasts sums to all 32 partitions of each group; DVE reciprocal +
     multiply; MM2 is 1 block-diag matmul per (b16, head-pair) reading
     ptn at partition bases {0,64} and a zero-padded vbdr tile loaded
     from DRAM.  O evicts as (hi, lo) fp8 pair packed uint16 (ACT hi,
     DVE lo) forming the DR-packed lhsT for phase C.
  C. out = O @ wo'^T/64 + bo, n-major with wo chunks streamed (no
     resident wo), 3-term fp8 DR like the other GEMMs.

All matmul accumulation in fp32 PSUM.  Measured rel RMS error vs the
fp32 reference: ~8.5e-3 (gate 2e-2).  PE cycle count ~1.65M (689 us at
2.4 GHz) vs 2.29M for the bf16 baseline; measured ~746 us.
"""

import os
import sys

for _p in ("/opt/trn_rl_repo", "/root/.axon_site/_ro/trn_rl_repo"):
    if os.path.isdir(_p) and _p not in sys.path:
        sys.path.append(_p)

import numpy as np
import ml_dtypes

import concourse.bass as bass
import concourse.mybir as mybir
import concourse.tile as tile
from concourse import bacc
from concourse import bass_utils

F32 = mybir.dt.float32
BF16 = mybir.dt.bfloat16
FP8 = mybir.dt.float8e4
U16 = mybir.dt.uint16
BF = ml_dtypes.bfloat16
E4M3 = ml_dtypes.float8_e4m3

N_CORES = 8
B, L, E, H, D = 512, 32, 2048, 32, 64
BS = B // N_CORES          # 64 batches per core
T = BS * L                 # 2048 tokens per core
P = 128
KT = E // P                # 16 contraction tiles (128)
CT = E // 256              # 8 contraction tiles (256, DoubleRow)
MT = T // P                # 16 row tiles
NT = 4                     # 512-wide output column tiles
NW = 512
SCALE = D ** -0.5
WS = 64.0                  # host weight prescale (folded out at eviction)
IWS = 1.0 / WS

HQ = H // 4                # 8 head-quad groups; 4 rounds (b0) each

Ident = mybir.ActivationFunctionType.Identity
Exp = mybir.ActivationFunctionType.Exp
DR = mybir.MatmulPerfMode.DoubleRow
MUL = mybir.AluOpType.mult
ADD = mybir.AluOpType.add

# (x-view, w-view) per compensation term: 0 = hi, 1 = lo
TERMS = ((0, 0), (0, 1), (1, 0))


def build_kernel(nc: bass.Bass, ph: int = 5):
    """ph: 1=A only, 2=+projections, 3=+MM1/exp, 4=+tails, 5=+C (full)."""
    f = nc.dram_tensor
    xpk_d = f("xpk", (T, E), U16, kind="ExternalInput").ap()
    wqp_d = f("wqp", (E, 2 * E), FP8, kind="ExternalInput").ap()
    wkp_d = f("wkp", (E, 2 * E), FP8, kind="ExternalInput").ap()
    wvp_d = f("wvp", (P, NT * 16384), FP8, kind="ExternalInput").ap()
    wop_d = f("wop", (P, NT * 16384), FP8, kind="ExternalInput").ap()
    bqk_d = f("bqk", (P, 2 * KT), F32, kind="ExternalInput").ap()
    bvf_d = f("bvf", (P, E), BF16, kind="ExternalInput").ap()
    bof_d = f("bof", (P, E), BF16, kind="ExternalInput").ap()
    relt_d = f("relt", (P, NW), BF16, kind="ExternalInput").ap()
    bonesb_d = f("bonesb", (P, P), BF16, kind="ExternalInput").ap()
    out_d = f("out", (T, E), F32, kind="ExternalOutput").ap()

    with tile.TileContext(nc) as tc:
        with (
            tc.tile_pool(name="dram", bufs=1, space="DRAM") as dram,
            tc.tile_pool(name="const", bufs=1) as const,
            tc.tile_pool(name="otp", bufs=1) as otp,
        ):
            relt = const.tile([P, NW], BF16)
            nc.gpsimd.dma_start(relt[:], relt_d[:])
            bqk = const.tile([P, 2 * KT], F32)
            nc.gpsimd.dma_start(bqk[:], bqk_d[:])
            bonesb = const.tile([P, P], BF16)
            nc.gpsimd.dma_start(bonesb[:], bonesb_d[:])

            # V in DRAM, laid out for block-diagonal round loads:
            # vd4[(hq//2)*128 + b0*32 + j, (hq%2)*4096 + hl*1024 + b16*64 + d]
            #   = V[b0*16+b16, j, 256*hq + 64*hl + d]
            vd4 = dram.tile([512, 8192], BF16)
            vd6 = vd4[:].rearrange(
                "(nn b0 j) (hq2 hl b4 pb d) -> nn b0 b4 pb j hq2 hl d",
                nn=NT, b0=4, j=32, hq2=2, hl=4, b4=4, pb=4, d=64)

            # O^T packed fp8 pair (byte0=hi, byte1=lo), DR-layout lhsT for C
            otu = otp.tile([P, KT * T], U16)
            otf = otu[:].bitcast(FP8).rearrange(
                "p (r t w) -> p r w t", r=KT, w=2)

            # Pools that live from phase A through phase C (attention tail
            # interleaves with early C blocks).  One shared PSUM pool (gps)
            # serves A's psv, B's pps and C's fps.
            with (
                tc.tile_pool(name="attp", bufs=1) as att_pool,
                tc.tile_pool(name="wqk", bufs=1) as wqk_pool,
                tc.tile_pool(name="gps", bufs=3, space="PSUM") as gps_p,
                tc.tile_pool(name="psb", bufs=1, space="PSUM") as psb_p,
                tc.tile_pool(name="pso", bufs=2, space="PSUM") as pso_p,
            ):
                stq = {}
                kbd = {}
                w_pre = {}
                init_cnt = {"kbd": 0, "vbd": 0}
                hold = {}

                def wload(which, m):
                    if (which, m) in w_pre:
                        return w_pre.pop((which, m))
                    wt = wqk_pool.tile([P, 2 * E], FP8, tag="w", bufs=3,
                                       name="wt")
                    nc.sync.dma_start(
                        wt[:],
                        (wqp_d if which == 0 else wkp_d)
                        [m * P:(m + 1) * P, :])
                    return wt

                def wprefetch(which, m):
                    w_pre[(which, m)] = wload(which, m)

                def mm1_round(hq, b0):
                    # vbdr[64*c2+32*u+j, b16*128 + u*64 + d] = V[b,j,...]
                    vbdr = att_pool.tile([P, 2048], BF16, tag="vbd",
                                         bufs=4, name="vbdr")
                    vb3 = vbdr[:].rearrange(
                        "p (b16 u d) -> p b16 u d", u=2, d=64)
                    if init_cnt["vbd"] < 4:
                        init_cnt["vbd"] += 1
                        nc.gpsimd.memset(vbdr[:], 0)
                    rbase = (hq // 2) * 128 + b0 * 32
                    for u in range(2):
                        for c2 in range(2):
                            hl = 2 * c2 + u
                            co = (hq % 2) * 4096 + hl * 1024
                            nc.scalar.dma_start(
                                vb3[64 * c2 + 32 * u:
                                    64 * c2 + 32 * u + 32, :, u, :],
                                vd4[rbase:rbase + 32, co:co + 1024]
                                .rearrange("j (b16 d) -> j b16 d", d=64),
                            )
                    pss = hold["pss_p"].tile([P, NW], F32, tag="ss",
                                             name="pss")
                    for b16 in range(16):
                        b = b0 * 16 + b16
                        for pi in range(2):
                            m = 2 * hq + pi
                            nc.tensor.matmul(
                                pss[64 * pi:64 * pi + 64,
                                    32 * b16:32 * b16 + 32],
                                kbd[m][:, b * 64:b * 64 + 64],
                                stq[m][:, b * L:(b + 1) * L],
                                start=True,
                                stop=True,
                                tile_position=(0, 64 * pi),
                            )
                    ptt = att_pool.tile([P, NW], BF16, tag="ptt",
                                        bufs=4, name="ptt")
                    nc.scalar.activation(ptt[:], pss[:], Exp, bias=0.0)
                    return ptt, vbdr

                def tails_head(hq, b0, rounds):
                    ptt, vbdr = rounds
                    psb = psb_p.tile([P, NW], F32, tag="bc", name="psb")
                    nc.tensor.matmul(psb[:], bonesb[:], ptt[:],
                                     start=True, stop=True)
                    rbc = att_pool.tile([P, NW], F32, tag="rbc",
                                        bufs=2, name="rbc")
                    nc.vector.reciprocal(rbc[:], psb[:])
                    ptn = att_pool.tile([P, NW], BF16, tag="ptn",
                                        bufs=2, name="ptn")
                    nc.vector.tensor_mul(ptn[:], ptt[:], rbc[:])
                    return ptn

                def tails_mm2(hq, b0, rounds, ptn):
                    ptt, vbdr = rounds
                    for c2 in range(2):
                        pso = pso_p.tile([P, NW], F32, tag="so",
                                         name="pso")
                        for b16 in range(16):
                            nc.tensor.matmul(
                                pso[:, 32 * b16:32 * b16 + 32],
                                vbdr[64 * c2:64 * c2 + 64,
                                     b16 * P:(b16 + 1) * P],
                                ptn[64 * c2:64 * c2 + 64,
                                    32 * b16:32 * b16 + 32],
                                start=True,
                                stop=True,
                                tile_position=(64 * c2, 0),
                            )
                        rt = 2 * hq + c2
                        hi = otf[:, rt, 0, b0 * NW:(b0 + 1) * NW]
                        lo = otf[:, rt, 1, b0 * NW:(b0 + 1) * NW]
                        with nc.allow_low_precision(
                            reason="fp8 hi/lo split keeps 2^-12 residual"
                        ):
                            nc.scalar.activation(hi, pso[:], Ident,
                                                 bias=0.0)
                            nc.vector.scalar_tensor_tensor(
                                lo, hi, -1.0, pso[:], op0=MUL, op1=ADD)

                def tails(hq, b0, rounds):
                    ptn = tails_head(hq, b0, rounds)
                    tails_mm2(hq, b0, rounds, ptn)

                tail_r = {}
                with tc.tile_pool(name="bigp", bufs=1) as bigp:
                    bigu = bigp.tile([P, KT * T], U16)
                    bigu3 = bigu[:].rearrange("p (k t) -> p k t", k=KT)
                    # fp8 views: [p][k 16][w 2][t 2048]
                    bigf = bigu[:].bitcast(FP8).rearrange(
                        "p (k t w) -> p k w t", k=KT, w=2)

                    # ---------------- Phase A ----------------
                    with tc.tile_pool(name="apool", bufs=1) as apool:
                        bvf = apool.tile([P, E], BF16)
                        nc.gpsimd.dma_start(bvf[:], bvf_d[:])

                        def wvload(n, nsplit=1):
                            wvn = apool.tile([P, 16384], FP8, tag="wv",
                                             bufs=2, name="wvn")
                            w = 16384 // nsplit
                            for h in range(nsplit):
                                nc.scalar.dma_start(
                                    wvn[:, h * w:(h + 1) * w],
                                    wvp_d[:, n * 16384 + h * w:
                                          n * 16384 + (h + 1) * w])
                            return wvn

                        def vproj(n, m, wvn):
                            wvn3 = wvn[:].rearrange(
                                "p (tw c i ncl) -> p tw c i ncl",
                                tw=2, c=CT, i=2)
                            psv = gps_p.tile([P, NW], F32, tag="g",
                                             name="psv")
                            idx = 0
                            for (xw, tw) in TERMS:
                                for c in range(CT):
                                    nc.tensor.matmul(
                                        psv[:],
                                        bigf[:, 2 * c:2 * c + 2, xw,
                                             m * P:(m + 1) * P],
                                        wvn3[:, tw, c],
                                        start=(idx == 0),
                                        stop=(idx == 3 * CT - 1),
                                        perf_mode=DR,
                                    )
                                    idx += 1
                            vev = apool.tile([P, NW], BF16, tag="vev",
                                             bufs=4, name="vev")
                            nc.vector.scalar_tensor_tensor(
                                vev[:], psv[:], IWS,
                                bvf[:, n * NW:(n + 1) * NW],
                                op0=MUL, op1=ADD,
                            )
                            # scatter (b0 = m//4, b16 = 4*(m%4) + p//32)
                            nc.gpsimd.dma_start(vd6[n, m // 4, m % 4],
                                                vev[:])

                        wvn0 = apool.tile([P, 16384], FP8, tag="wv",
                                          bufs=2, name="wvn0")
                        for tt in range(3):
                            nc.sync.dma_start_transpose(
                                bigu3[:, :, tt * P:(tt + 1) * P],
                                xpk_d[tt * P:(tt + 1) * P, :])
                            h2 = slice(tt * 4096, (tt + 1) * 4096)
                            nc.scalar.dma_start(wvn0[:, h2], wvp_d[:, h2])
                        nc.scalar.dma_start(wvn0[:, 12288:16384],
                                            wvp_d[:, 12288:16384])
                        wvn1 = apool.tile([P, 16384], FP8, tag="wv", bufs=2,
                                          name="wvn1")
                        # interleave n=0 and n=1 passes: the transpose DMA
                        # deficit amortizes over twice the PE work
                        OFF = 6
                        for m in range(MT):
                            if m < 4:
                                nc.scalar.dma_start(
                                    wvn1[:, m * 4096:(m + 1) * 4096],
                                    wvp_d[:, 16384 + m * 4096:
                                          16384 + (m + 1) * 4096])
                            if m + 3 < MT:
                                nc.sync.dma_start_transpose(
                                    bigu3[:, :, (m + 3) * P:(m + 4) * P],
                                    xpk_d[(m + 3) * P:(m + 4) * P, :])
                            vproj(0, m, wvn0)
                            if m >= OFF:
                                vproj(1, m - OFF, wvn1)
                        wvn2 = apool.tile([P, 16384], FP8, tag="wv",
                                          bufs=2, name="wvn2")
                        for m in range(MT - OFF, MT):
                            if m - (MT - OFF) < 4:
                                h = m - (MT - OFF)
                                nc.scalar.dma_start(
                                    wvn2[:, h * 4096:(h + 1) * 4096],
                                    wvp_d[:, 32768 + h * 4096:
                                          32768 + (h + 1) * 4096])
                            vproj(1, m, wvn1)
                        wprefetch(0, 0)
                        wvn3 = apool.tile([P, 16384], FP8, tag="wv",
                                          bufs=2, name="wvn3")
                        for m in range(MT):
                            if m < 4:
                                nc.scalar.dma_start(
                                    wvn3[:, m * 4096:(m + 1) * 4096],
                                    wvp_d[:, 49152 + m * 4096:
                                          49152 + (m + 1) * 4096])
                            vproj(2, m, wvn2)
                        for m in range(MT):
                            vproj(3, m, wvn3)
                        wprefetch(1, 0)

                    # ---------------- Phase B ----------------
                    with (
                        tc.tile_pool(name="stq", bufs=1) as stq_pool,
                        tc.tile_pool(name="kbd", bufs=1) as kbd_pool,
                        tc.tile_pool(name="pss", bufs=2,
                                     space="PSUM") as pss_p,
                    ):
                        hold["pss_p"] = pss_p

                        def proj(which, m):
                            wt = wload(which, m)
                            wt3 = wt[:].rearrange(
                                "p (tw c i mc) -> p tw c i mc",
                                tw=2, c=CT, i=2)
                            if which == 0:
                                st = stq_pool.tile([P, T], BF16, tag="stq",
                                                   bufs=3, name="st")
                                stq[m] = st
                            else:
                                kb = kbd_pool.tile([P, 64 * BS], BF16,
                                                   tag="kbd", bufs=3,
                                                   name="kb")
                                kbd[m] = kb
                                kb3 = kb[:].rearrange("p (b z) -> p b z",
                                                      b=BS)
                                if init_cnt["kbd"] < 3:
                                    init_cnt["kbd"] += 1
                                    nc.gpsimd.memset(kb[:], 0)
                            for n in range(NT):
                                pps = gps_p.tile([P, NW], F32, tag="g",
                                                 name="pps")
                                idx = 0
                                for (xw, tw) in TERMS:
                                    for c in range(CT):
                                        nc.tensor.matmul(
                                            pps[:],
                                            wt3[:, tw, c],
                                            bigf[:, 2 * c:2 * c + 2, xw,
                                                 n * NW:(n + 1) * NW],
                                            start=(idx == 0),
                                            stop=(idx == 3 * CT - 1),
                                            perf_mode=DR,
                                        )
                                        idx += 1
                                bs = slice(16 * n, 16 * (n + 1))
                                if which == 0:
                                    nc.scalar.activation(
                                        st[:, n * NW:(n + 1) * NW], pps[:],
                                        Ident, bias=bqk[:, m:m + 1],
                                        scale=IWS)
                                else:
                                    # scatter-evict into block-diag kbd
                                    for hh in range(2):
                                        pr = slice(64 * hh, 64 * hh + 64)
                                        dst = kb3[pr, bs,
                                                  32 * hh:32 * hh + 32]
                                        nc.scalar.activation(
                                            dst,
                                            pps[pr, :].rearrange(
                                                "p (b j) -> p b j", j=L),
                                            Ident,
                                            bias=bqk[pr,
                                                     KT + m:KT + m + 1],
                                            scale=IWS)
                                        rel3 = (relt[pr,
                                                     m * L:(m + 1) * L]
                                                .unsqueeze(1)
                                                .broadcast_to([64, 16, L]))
                                        nc.vector.tensor_add(dst, dst,
                                                             rel3)

                        def proj_pair(g):
                            prev = []
                            if g > 0:
                                prev = [(g - 1, b0) for b0 in range(4)]
                            rnd = [None] * 4
                            wprefetch(0, 2 * g)
                            wprefetch(1, 2 * g)
                            if ph >= 2:
                                proj(0, 2 * g)
                            wprefetch(0, 2 * g + 1)
                            if prev and ph >= 3:
                                rnd[0] = mm1_round(*prev[0])
                            if ph >= 2:
                                proj(1, 2 * g)
                            wprefetch(1, 2 * g + 1)
                            if prev:
                                if ph >= 4:
                                    tails(*prev[0], rnd[0])
                                if ph >= 3:
                                    rnd[1] = mm1_round(*prev[1])
                            if ph >= 2:
                                proj(0, 2 * g + 1)
                            if prev:
                                if ph >= 4:
                                    tails(*prev[1], rnd[1])
                                if ph >= 3:
                                    rnd[2] = mm1_round(*prev[2])
                            if ph >= 2:
                                proj(1, 2 * g + 1)
                            if prev:
                                if ph >= 4:
                                    tails(*prev[2], rnd[2])
                                if ph >= 3:
                                    rnd[3] = mm1_round(*prev[3])
                                    if ph >= 4:
                                        tails(*prev[3], rnd[3])

                        for g in range(HQ):
                            proj_pair(g)

                        # att group 7: MM1 rounds here (pss still open);
                        # tails interleave with early phase-C blocks
                        if ph >= 3:
                            for b0 in range(4):
                                tail_r[b0] = mm1_round(7, b0)

                # ---------------- Phase C ----------------
                with tc.tile_pool(name="cpool", bufs=1) as cpool:
                    bof = cpool.tile([P, E], BF16)
                    nc.gpsimd.dma_start(bof[:], bof_d[:])

                    def woload(n, nsplit=1):
                        won = cpool.tile([P, 16384], FP8, tag="wo", bufs=2,
                                         name="won")
                        w = 16384 // nsplit
                        for h in range(nsplit):
                            nc.scalar.dma_start(
                                won[:, h * w:(h + 1) * w],
                                wop_d[:, n * 16384 + h * w:
                                      n * 16384 + (h + 1) * w])
                        return won

                    def cblock(n, m, won3, mid=None):
                        fps = gps_p.tile([P, NW], F32, tag="g", name="fps")
                        idx = 0
                        for (xw, tw) in TERMS:
                            for c in range(CT):
                                nc.tensor.matmul(
                                    fps[:],
                                    otf[:, 2 * c:2 * c + 2, xw,
                                        m * P:(m + 1) * P],
                                    won3[:, tw, c],
                                    start=(idx == 0),
                                    stop=(idx == 3 * CT - 1),
                                    perf_mode=DR,
                                )
                                idx += 1
                        fout = cpool.tile([P, NW], F32, tag="fo", bufs=3,
                                          name="fout")
                        pieces = 2 if (n == NT - 1 and m == MT - 1) else 1
                        w = NW // pieces
                        for h in range(pieces):
                            cs = slice(h * w, (h + 1) * w)
                            nc.vector.scalar_tensor_tensor(
                                fout[:, cs], fps[:, cs], IWS,
                                bof[:, n * NW + h * w:n * NW + (h + 1) * w],
                                op0=MUL, op1=ADD)
                            nc.sync.dma_start(
                                out_d[m * P:(m + 1) * P,
                                      n * NW + h * w:n * NW + (h + 1) * w],
                                fout[:, cs])

                    tail_h = {}
                    if ph >= 5:
                        won = woload(0, nsplit=4)
                        wonn = woload(1)
                        for n in range(NT):
                            won3 = won[:].rearrange(
                                "p (tw c i ncl) -> p tw c i ncl",
                                tw=2, c=CT, i=2)
                            for m in range(MT):
                                # att7 tails interleave with early C
                                if n == 0 and m < 4 and ph >= 4:
                                    tails(7, m, tail_r[m])
                                cblock(n, m, won3)
                            won = wonn
                            if n + 2 < NT:
                                wonn = woload(n + 2)
                    elif ph >= 4:
                        for b0 in range(4):
                            tails(7, b0, tail_r[b0])
    return nc


def host_prep(wq, bq, wk, bk, wv, bv, wo, bo, rel_pos_enc):
    """Shared (core-replicated) input tensors, laid out for the kernel."""
    def split8(w):
        hi = w.astype(E4M3)
        lo = (w - hi.astype(np.float32)).astype(E4M3)
        return hi, lo

    def pack_lhst(w):
        # w: [e_out, e_in]; returns (E, 2E) fp8:
        # wp[m*128+p, tw*2048 + c*256 + i*128 + mc] = Wtw^T[256c+128i+p, 128m+mc]
        hi, lo = split8(WS * w)
        wt = np.stack([hi.T, lo.T])           # [tw, e_in, e_out]
        tmp = np.ascontiguousarray(
            wt.reshape(2, CT, 2, P, KT, P).transpose(4, 3, 0, 1, 2, 5))
        return tmp.reshape(E, 2 * E)

    def pack_rhs(w):
        # w: [e_out, e_in]; returns (128, NT*16384) fp8:
        # wp[p, n*16384 + tw*8192 + c*1024 + i*512 + ncl]
        #   = Wtw^T[256c+128i+p, 512n+ncl]
        hi, lo = split8(WS * w)
        wt = np.stack([hi.T, lo.T])           # [tw, e_in, e_out]
        tmp = np.ascontiguousarray(
            wt.reshape(2, CT, 2, P, NT, NW).transpose(3, 4, 0, 1, 2, 5))
        return tmp.reshape(P, NT * 16384)

    wqp = pack_lhst(wq)
    wkp = pack_lhst(SCALE * wk)
    wvp = pack_rhs(wv)
    wop = pack_rhs(wo)

    bqk = np.zeros((P, 2 * KT), np.float32)
    bqk[:, :KT] = bq.reshape(KT, P).T
    bqk[:, KT:] = (SCALE * bk).reshape(KT, P).T
    bvf = np.ascontiguousarray(np.broadcast_to(bv, (P, E))).astype(BF)
    bof = np.ascontiguousarray(np.broadcast_to(bo, (P, E))).astype(BF)

    # relT[64c+d, 32m+j] = rel_pos_enc[(2m+c) - j + 31, d]
    relt = np.zeros((P, NW), np.float32)
    j = np.arange(L)
    for m in range(KT):
        for c in range(2):
            h = 2 * m + c
            blk = rel_pos_enc[h - j + (L - 1), :]        # [j, d]
            relt[64 * c:64 * c + 64, 32 * m:32 * m + 32] = blk.T
    relt = relt.astype(BF)

    bonesb = np.zeros((P, P), np.float32)
    for g in range(4):
        bonesb[32 * g:32 * g + 32, 32 * g:32 * g + 32] = 1
    bonesb = bonesb.astype(BF)

    return dict(
        wqp=wqp, wkp=wkp, wvp=wvp, wop=wop, bqk=bqk, bvf=bvf, bof=bof,
        relt=relt, bonesb=bonesb,
    )


def pack_x(x_core):
    # x_core: [T, E] f32 -> packed uint16 (byte0 = hi, byte1 = lo)
    hi = x_core.astype(E4M3)
    lo = (x_core - hi.astype(np.float32)).astype(E4M3)
    return (hi.view(np.uint8).astype(np.uint16)
            | (lo.view(np.uint8).astype(np.uint16) << 8))


_CACHE = {}


def _get_nc():
    if "nc" not in _CACHE:
        nc = bacc.Bacc(
            "TRN2",
            target_bir_lowering=False,
            debug=False,
            enable_asserts=False,
            num_devices=N_CORES,
        )
        build_kernel(nc, ph=int(os.environ.get("KPH", "5")))
        nc.compile()
        _CACHE["nc"] = nc
    return _CACHE["nc"]


def kernel(x, wq, bq, wk, bk, wv, bv, wo, bo, rel_pos_enc, _return_maps=False):
    x = np.asarray(x, dtype=np.float32)
    shared = host_prep(
        np.asarray(wq, np.float32), np.asarray(bq, np.float32),
        np.asarray(wk, np.float32), np.asarray(bk, np.float32),
        np.asarray(wv, np.float32), np.asarray(bv, np.float32),
        np.asarray(wo, np.float32), np.asarray(bo, np.float32),
        np.asarray(rel_pos_enc, np.float32),
    )
    in_maps = []
    for c in range(N_CORES):
        m = dict(shared)
        m["xpk"] = pack_x(
            np.ascontiguousarray(x[c * BS:(c + 1) * BS].reshape(T, E)))
        in_maps.append(m)
    if _return_maps:
        return in_maps

    nc = _get_nc()
    res = bass_utils.run_bass_kernel_spmd(
        nc, in_maps, core_ids=list(range(N_CORES)), trace=False
    )
    out = np.concatenate(
        [res.results[c]["out"].reshape(BS, L, E) for c in range(N_CORES)],
        axis=0,
    )
    return out.astype(np.float32)


if __name__ == "__main__":
    rng = np.random.default_rng(0)
    ins = {
        "x": rng.standard_normal((B, L, E), dtype=np.float32),
        "wq": rng.standard_normal((E, E), dtype=np.float32) * 0.02,
        "bq": np.zeros(E, np.float32),
        "wk": rng.standard_normal((E, E), dtype=np.float32) * 0.02,
        "bk": np.zeros(E, np.float32),
        "wv": rng.standard_normal((E, E), dtype=np.float32) * 0.02,
        "bv": np.zeros(E, np.float32),
        "wo": rng.standard_normal((E, E), dtype=np.float32) * 0.02,
        "bo": np.zeros(E, np.float32),
        "rel_pos_enc": rng.standard_normal((2 * L - 1, D),
                                           dtype=np.float32),
    }

    def np_ref(x, wq, bq, wk, bk, wv, bv, wo, bo, rel_pos_enc):
        b, l, e = x.shape
        h, d = H, D
        q = (x @ wq.T + bq).reshape(b, l, h, d).transpose(0, 2, 1, 3)
        k = (x @ wk.T + bk).reshape(b, l, h, d).transpose(0, 2, 1, 3)
        v = (x @ wv.T + bv).reshape(b, l, h, d).transpose(0, 2, 1, 3)
        scores = np.einsum("bhid,bhjd->bhij", q, k) * (d ** -0.5)
        pos = np.arange(l)
        rel_idx = pos[:, None] - pos[None, :] + (l - 1)
        rel_pos = rel_pos_enc[rel_idx]
        scores = scores + np.einsum("bhid,hjd->bhij", q, rel_pos)
        scores -= scores.max(axis=-1, keepdims=True)
        attn = np.exp(scores)
        attn /= attn.sum(axis=-1, keepdims=True)
        out = np.einsum("bhij,bhjd->bhid", attn, v)
        out = out.transpose(0, 2, 1, 3).reshape(b, l, e)
        return out @ wo.T + bo

    nb = int(os.environ.get("KNB", "64"))  # batches to check
    out = kernel(**ins)
    exp = np_ref(**{k: (v[:nb] if k == "x" else v) for k, v in ins.items()})
    err = out[:nb] - exp
    rel = np.linalg.norm(err) / np.linalg.norm(exp)
    print(f"kernel out: {out.shape} rel err (first {nb} batches): {rel:.6e}")
